# revision 35
# baseline (speedup 1.0000x reference)
"""DetectionBoxLoss (masked CIoU loss) on 8 TRN2 NeuronCores.

Strategy: pure data-parallel over the batch dim (64 -> 8 per core); each core
reduces its shard to per-partition (loss_sum, mask_count) partials, the host
sums 16 numbers and divides.

Math: with boxes decoded as x1 = cx - l*s, x2 = cx + r*s (same grid for pred
and target), the grid cx/cy cancels in every CIoU term, so the whole
computation reduces to elementwise ops on the 8 reg channels (stride factors
cancel inside the iou / rho2/c2 / aspect ratios too):
    w1 = l1+r1, h1 = t1+b1, w2 = l2+r2, h2 = t2+b2        (units of stride)
    iw = min(l1,l2)+min(r1,r2), ih = min(t1,t2)+min(b1,b2)
    cw = (w1+w2)-iw, ch = (h1+h2)-ih
    inter = iw*ih ; union = w1*h1 + w2*h2 - inter ; iou = inter/union
    rho2/c2 = 0.25*(dx^2+dy^2)/(cw^2+ch^2), dx = (r2-l2)-(r1-l1), dy likewise
    dv = atan(w2/h2)-atan(w1/h1) = atan2(N, D), N = w2*h1-w1*h2, D = h1*h2+w1*w2
    v = (4/pi^2)*dv^2 ; alpha*v = v^2/(v - iou + 1 + eps)
    loss = (sum(m) + sum((rho2/c2 + alpha*v - iou)*m)) / max(sum(m), 1)
atan on ScalarE only covers [-pi/2, pi/2], so atan2(|N|,D) is computed as
theta = atan(min/max) folded with p = (|N| > D):  dv^2 = (p*pi/2 - theta)^2
via z = p - (2/pi)*theta, v = z^2.

Implementation: RAW bass (explicit per-engine streams + semaphores) because
this toolchain allows at most ONE semaphore wait per instruction: waits are
standalone wait_ge ops, producers carry then_inc updates. The host pre-tiles
the inputs so each (iteration, axis-set) is one fully-contiguous DMA: X tile
columns = [l1|r1|l2|r2], Y = [t1|b1|t2|b2]; the u8 mask is cast to f32 by a
gpsimd (SWDGE) DMA.
"""

import math
import sys

import numpy as np

sys.path.insert(0, "/opt/trn_rl_repo")

from concourse import bass, mybir  # noqa: E402
from concourse.bass_utils import run_bass_kernel_spmd  # noqa: E402

EPS = 1e-7
N_CORES = 8
B, C, H, W = 64, 4, 192, 192
BPC = B // N_CORES  # batch per core
PIX = H * W  # 36864
FREE = BPC * PIX // 128  # 2304 free elems per partition per channel per core

F32 = mybir.dt.float32
U8 = mybir.dt.uint8
Alu = mybir.AluOpType
Act = mybir.ActivationFunctionType

_CACHE = {}


def _build(n_tiles: int, cdt):
    F = FREE // n_tiles
    nc = bass.Bass(detect_race_conditions=False)
    regs = nc.declare_dram_parameter(
        "regs", [2, n_tiles, BPC, 16, 4, F], F32, isOutput=False
    )
    mask = nc.declare_dram_parameter(
        "mask", [n_tiles, BPC, 16, F], U8, isOutput=False
    )
    out = nc.declare_dram_parameter("out", [2, 128], F32, isOutput=True)
    regs_ap = regs[:]
    mask_ap = mask[:]

    X = [nc.alloc_sbuf_tensor(f"X{t}", [128, 4 * F], F32).ap() for t in range(n_tiles)]
    Y = [nc.alloc_sbuf_tensor(f"Y{t}", [128, 4 * F], F32).ap() for t in range(n_tiles)]
    MK = [nc.alloc_sbuf_tensor(f"MK{t}", [128, F], U8).ap() for t in range(n_tiles)]
    d = [nc.alloc_sbuf_tensor(f"d{i}", [128, F], cdt).ap() for i in range(11)]
    A = [nc.alloc_sbuf_tensor(f"A{i}", [128, F], cdt).ap() for i in range(7)]
    lacc = [
        nc.alloc_sbuf_tensor(f"lacc{t}", [128, 1], F32).ap() for t in range(n_tiles)
    ]
    macc = [
        nc.alloc_sbuf_tensor(f"macc{t}", [128, 1], F32).ap() for t in range(n_tiles)
    ]
    part = nc.alloc_sbuf_tensor("part", [128, 1], F32).ap()
    partm = nc.alloc_sbuf_tensor("partm", [128, 1], F32).ap()

    import contextlib

    with contextlib.ExitStack() as ctx:
        block = ctx.enter_context(nc.Block())
        xy_sem = [
            ctx.enter_context(nc.semaphore(f"xy{t}")) for t in range(n_tiles)
        ]
        mk_sem = [
            ctx.enter_context(nc.semaphore(f"mk{t}")) for t in range(n_tiles)
        ]
        d2a = ctx.enter_context(nc.semaphore("d2a"))
        a2d = ctx.enter_context(nc.semaphore("a2d"))
        fin = ctx.enter_context(nc.semaphore("fin"))
        odma = ctx.enter_context(nc.semaphore("odma"))
        rsem = ctx.enter_context(nc.semaphore("rsem"))

        @block.sync
        def _(sp):
            for t in range(n_tiles):
                sp.dma_start(out=X[t], in_=regs_ap[0, t]).then_inc(xy_sem[t], 16)
                sp.dma_start(out=Y[t], in_=regs_ap[1, t]).then_inc(xy_sem[t], 16)
                sp.dma_start(out=MK[t], in_=mask_ap[t]).then_inc(mk_sem[t], 16)
            sp.wait_ge(fin, 1)
            sp.dma_start(out=out[:][0], in_=lacc[n_tiles - 1]).then_inc(odma, 16)
            sp.dma_start(out=out[:][1], in_=macc[n_tiles - 1]).then_inc(odma, 16)
            sp.wait_ge(odma, 32)

        @block.gpsimd
        def _(gp):
            pass

        @block.scalar
        def _(act):
            for t in range(n_tiles):
                k = 6 * t
                act.wait_ge(d2a, k + 1)
                act.activation(A[0], d[5], Act.Square).then_inc(a2d)   # cwq
                act.wait_ge(d2a, k + 2)
                act.activation(A[1], d[2], Act.Square).then_inc(a2d)   # chq
                act.wait_ge(d2a, k + 3)
                act.activation(A[2], d[9], Act.Square, scale=0.5).then_inc(a2d)
                act.wait_ge(d2a, k + 4)
                act.activation(A[3], d[10], Act.Square, scale=0.5).then_inc(a2d)
                act.wait_ge(d2a, k + 5)
                act.activation(A[4], d[7], Act.Arctan).then_inc(a2d)   # theta
                act.wait_ge(d2a, k + 6)
                act.activation(A[5], d[0], Act.Square).then_inc(a2d)   # vv = v
                act.activation(A[6], A[5], Act.Square).then_inc(a2d)   # v^2

        @block.vector
        def _(v):
            for t in range(n_tiles):
                Xt, Yt, mkf = X[t], Y[t], MK[t]
                l1, r1 = Xt[:, 0:F], Xt[:, F:2 * F]
                l2, r2 = Xt[:, 2 * F:3 * F], Xt[:, 3 * F:4 * F]
                t1, b1 = Yt[:, 0:F], Yt[:, F:2 * F]
                t2, b2 = Yt[:, 2 * F:3 * F], Yt[:, 3 * F:4 * F]
                j = 7 * t

                v.wait_ge(xy_sem[t], 32)
                v.tensor_tensor(d[0], l1, l2, Alu.min)            # Lm
                v.tensor_add(d[1], l1, r1)                        # w1
                v.tensor_tensor(d[2], r1, r2, Alu.min)            # Rm
                v.tensor_tensor(d[3], t1, t2, Alu.min)            # Tm
                v.tensor_add(d[4], t1, b1)                        # h1
                v.tensor_tensor(d[5], b1, b2, Alu.min)            # Bm
                v.tensor_add(d[6], l2, r2)                        # w2
                v.tensor_add(d[7], t2, b2)                        # h2
                v.tensor_add(d[8], d[0], d[2])                    # iw
                v.tensor_add(d[0], d[3], d[5])                    # ih
                v.tensor_add(d[2], d[1], d[6])                    # Sw
                v.tensor_add(d[3], d[4], d[7])                    # Sh
                v.tensor_sub(d[5], d[2], d[8]).then_inc(d2a)      # cw     6t+1
                v.tensor_sub(d[2], d[3], d[0]).then_inc(d2a)      # ch     6t+2
                v.tensor_mul(d[3], d[8], d[0])                    # inter
                v.tensor_sub(d[8], r2, r1)                        # dxa
                v.tensor_sub(d[0], l2, l1)                        # dxb
                v.tensor_sub(d[9], d[8], d[0]).then_inc(d2a)      # dx     6t+3
                v.tensor_sub(d[8], b2, b1)                        # dya
                v.tensor_sub(d[0], t2, t1)                        # dyb
                v.tensor_sub(d[10], d[8], d[0]).then_inc(d2a)     # dy     6t+4
                v.wait_ge(a2d, j + 2)
                v.tensor_add(d[8], A[0], A[1])                    # c2
                v.wait_ge(a2d, j + 4)
                v.tensor_add(d[0], A[2], A[3])                    # rho2q
                v.tensor_mul(d[2], d[1], d[4])                    # a1
                v.tensor_mul(d[5], d[6], d[7])                    # a2
                v.tensor_add(d[9], d[2], d[5])                    # s12
                v.tensor_sub(d[2], d[9], d[3])                    # U
                v.reciprocal(d[9], d[2])                          # 1/U
                v.tensor_mul(d[5], d[3], d[9])                    # iou
                v.reciprocal(d[2], d[8])                          # 1/c2
                v.tensor_mul(d[3], d[0], d[2])                    # rterm
                v.tensor_mul(d[0], d[6], d[4])                    # n1 = w2*h1
                v.tensor_mul(d[2], d[1], d[7])                    # n2 = w1*h2
                v.tensor_sub(d[8], d[0], d[2])                    # Nt
                v.tensor_mul(d[0], d[4], d[7])                    # h1*h2
                v.tensor_mul(d[2], d[1], d[6])                    # w1*w2
                v.tensor_add(d[1], d[0], d[2])                    # Dt
                v.tensor_scalar_mul(d[0], d[8], -1.0)             # -Nt
                v.tensor_max(d[0], d[8], d[0])                    # |Nt|
                v.tensor_tensor(d[2], d[0], d[1], Alu.min)        # mn
                v.tensor_tensor(d[4], d[0], d[1], Alu.max)        # mx
                v.reciprocal(d[8], d[4])                          # 1/mx
                v.tensor_mul(d[7], d[2], d[8]).then_inc(d2a)      # ratio  6t+5
                v.tensor_tensor(d[2], d[0], d[1], Alu.is_gt)      # p
                v.wait_ge(a2d, j + 5)
                v.tensor_scalar_mul(d[4], A[4], -2.0 / math.pi)   # -2*theta/pi
                v.tensor_add(d[0], d[4], d[2]).then_inc(d2a)      # z1     6t+6
                v.wait_ge(a2d, j + 6)
                v.tensor_scalar_add(d[4], A[5], 1.0 + EPS)        # v + 1 + eps
                v.tensor_sub(d[1], d[4], d[5])                    # dn2
                v.wait_ge(a2d, j + 7)
                v.reciprocal(d[9], d[1])                          # 1/dn2
                v.tensor_mul(d[2], A[6], d[9])                    # alpha*v
                v.tensor_add(d[4], d[3], d[2])                    # s1
                v.tensor_sub(d[1], d[4], d[5])                    # u2
                v.wait_ge(mk_sem[t], 16)
                v.tensor_mul(d[2], d[1], mkf)                     # u2*m
                v.tensor_copy(out=d[3], in_=mkf)                  # m as f32
                # [128,1] reduce writebacks land late; fence with a self-sem
                # before any same-engine consumer reads them.
                if t == 0:
                    v.tensor_reduce(
                        lacc[0], d[2], mybir.AxisListType.X, Alu.add
                    ).then_inc(rsem)
                    v.tensor_reduce(
                        macc[0], d[3], mybir.AxisListType.X, Alu.add
                    ).then_inc(rsem)
                else:
                    v.tensor_reduce(
                        part, d[2], mybir.AxisListType.X, Alu.add
                    ).then_inc(rsem)
                    v.tensor_reduce(
                        partm, d[3], mybir.AxisListType.X, Alu.add
                    ).then_inc(rsem)
                    v.wait_ge(rsem, 3 * t + 1)
                    v.tensor_add(lacc[t], lacc[t - 1], part)
                    v.tensor_add(macc[t], macc[t - 1], partm).then_inc(rsem)
                if t == n_tiles - 1:
                    v.wait_ge(rsem, 3 * n_tiles - 1 if n_tiles > 1 else 2).then_inc(fin)
    return nc


def _get_nc(n_tiles=2, cdt=F32):
    key = (n_tiles, str(cdt))
    if key not in _CACHE:
        _CACHE[key] = _build(n_tiles, cdt)
    return _CACHE[key]


def _host_layout(pred_reg, target_reg, mask, n_tiles):
    F = FREE // n_tiles
    hql = F // W
    arr = np.stack(
        [
            np.asarray(pred_reg, dtype=np.float32),
            np.asarray(target_reg, dtype=np.float32),
        ],
        axis=1,
    )  # [b, g, j, H, W]; j = jj*2 + j2
    arr = arr.reshape(B, 2, 2, 2, 16, n_tiles, hql, W)  # b g jj j2 hp t hql w
    arr = arr.transpose(3, 0, 5, 4, 1, 2, 6, 7).reshape(2, B, n_tiles, 16, 4, F)
    mk = np.asarray(mask).view(np.uint8).reshape(B, 16, n_tiles, hql, W)
    mk = mk.transpose(0, 2, 1, 3, 4).reshape(B, n_tiles, 16, F)
    return arr, mk


def run(pred_reg, target_reg, mask, trace=False, n_tiles=2, cdt=F32):
    regs, mask_u8 = _host_layout(pred_reg, target_reg, mask, n_tiles)
    nc = _get_nc(n_tiles, cdt)
    in_maps = []
    for i in range(N_CORES):
        sl = slice(i * BPC, (i + 1) * BPC)
        in_maps.append(
            {
                "regs": np.ascontiguousarray(regs[:, sl].transpose(0, 2, 1, 3, 4, 5)),
                "mask": np.ascontiguousarray(mask_u8[sl].transpose(1, 0, 2, 3)),
            }
        )
    res = run_bass_kernel_spmd(nc, in_maps, core_ids=list(range(N_CORES)), trace=trace)
    parts = np.stack([r["out"] for r in res.results])  # [8, 2, 128]
    lsum = float(parts[:, 0, :].sum())
    msum = float(parts[:, 1, :].sum())
    loss = np.float32((msum + lsum) / max(msum, 1.0))
    return np.asarray(loss, dtype=np.float32), res


def kernel(pred_reg, target_reg, mask):
    loss, _ = run(pred_reg, target_reg, mask)
    return loss
